# revision 4
# baseline (speedup 1.0000x reference)
"""Trainium2 Bass kernel for nn_MultiHeadNetwork (moe_routing).

Strategy
--------
Host side (numpy, inside kernel()):
  * task id per row = argmax of the trailing one-hot block of x (data, not
    activation dependent), rows sorted by task id, batch split into 8
    contiguous 512-row chunks (one per NeuronCore).
  * Trunk weights replicated; per core the head only needs the few tasks its
    sorted chunk spans (<= S slots, S = max over cores), gathered on host.
  * All tensors are pre-packed so every DMA is a contiguous [128, F] panel.

Device side (one SPMD Tile program on 8 cores):
  * Activations kept feature-major (hT: [feat partitions, batch free]) so each
    trunk layer is out = W_chunk.T @ hT with NO transposes anywhere:
       lhsT = W[kc, wc] 128x128 (stationary), rhs = hT k-tile [128, 512].
  * Matmuls run as float32r (fp32 storage, full-rate PE for N >= 256).
  * ReLU + bias fused on the scalar engine straight out of PSUM.
  * Head: for each task slot s compute outT_s = head_W[t_s].T @ h3T for ALL
    512 columns (N=512 keeps the PE at full rate), then select the columns
    whose task == t_s via copy_predicated with a host-built 0/1 mask.
  * Head bias and the inverse permutation are applied on host.
"""

import numpy as np
from contextlib import ExitStack

import concourse.bacc as bacc
import concourse.mybir as mybir
from concourse.tile import TileContext
from concourse import bass_utils

BATCH = 4096
FEAT = 2048
NUM_TASKS = 50
WIDTH = 2048
HEAD_DIM = 256
NCORES = 8
BPC = BATCH // NCORES          # 512 rows per core
KIN = FEAT + NUM_TASKS         # 2098
KC0 = 17                       # input K chunks (zero-padded to 2176)
KPAD = KC0 * 128
KC = WIDTH // 128              # 16
WC = WIDTH // 128              # 16
MH = HEAD_DIM // 128           # 2 head-dim halves

F32 = mybir.dt.float32
F32R = mybir.dt.float32r

_PROG_CACHE: dict = {}


def round_fp32r(a: np.ndarray) -> np.ndarray:
    """Round fp32 to the fp32r grid (11 mantissa bits, RNE) like the HW does."""
    b = np.ascontiguousarray(a, np.float32).view(np.uint32)
    bias = np.uint32(0x7FF) + ((b >> np.uint32(12)) & np.uint32(1))
    out = (b + bias) & np.uint32(0xFFFFF000)
    return out.view(np.float32)


def _build(S: int):
    """Build + compile the SPMD Tile program for S head slots per core."""
    nc = bacc.Bacc("TRN2", target_bir_lowering=False, debug=False)
    xT = nc.dram_tensor("xT", [KC0, 128, BPC], F32R, kind="ExternalInput").ap()
    w0 = nc.dram_tensor("w0p", [WC, 128, KC0 * 128], F32R, kind="ExternalInput").ap()
    w1 = nc.dram_tensor("w1p", [WC, 128, KC * 128], F32R, kind="ExternalInput").ap()
    w2 = nc.dram_tensor("w2p", [WC, 128, KC * 128], F32R, kind="ExternalInput").ap()
    bia = nc.dram_tensor("bias", [128, 3 * WC], F32, kind="ExternalInput").ap()
    hws = nc.dram_tensor("hws", [S, 128, KC * HEAD_DIM], F32R, kind="ExternalInput").ap()
    msk = nc.dram_tensor("msk", [128, S * BPC], mybir.dt.uint8, kind="ExternalInput").ap()
    out = nc.dram_tensor("outT", [MH, 128, BPC], F32, kind="ExternalOutput").ap()

    with TileContext(nc) as tc, ExitStack() as ctx:
        # xT (17 tiles) and h2 (16) share slots: h2 allocates only after
        # layer 0 fully finished reading xT.  Same for h1 vs nothing; h3 gets
        # its own pool (h1 is still being read while h3 is produced).
        actA = ctx.enter_context(tc.tile_pool(name="actA", bufs=KC0))
        actB = ctx.enter_context(tc.tile_pool(name="actB", bufs=KC))
        actC = ctx.enter_context(tc.tile_pool(name="actC", bufs=KC))
        wp = ctx.enter_context(tc.tile_pool(name="wp", bufs=3))
        cons = ctx.enter_context(tc.tile_pool(name="cons", bufs=1))
        hwp = ctx.enter_context(tc.tile_pool(name="hwp", bufs=2))
        op = ctx.enter_context(tc.tile_pool(name="op", bufs=MH))
        psA = ctx.enter_context(tc.tile_pool(name="psA", bufs=4, space="PSUM"))
        psB = ctx.enter_context(tc.tile_pool(name="psB", bufs=4, space="PSUM"))

        bt = cons.tile([128, 3 * WC], F32, tag="bt")
        nc.sync.dma_start(bt[:], bia)
        mt = cons.tile([128, S * BPC], mybir.dt.uint8, tag="mt")
        nc.sync.dma_start(mt[:], msk)

        xt = []
        for k in range(KC0):
            t = actA.tile([128, BPC], F32R, tag="actA")
            nc.sync.dma_start(t[:], xT[k])
            xt.append(t)

        def trunk_layer(src, wdram, nk, li, pool, tag):
            outs = []
            for w in range(WC):
                wt = wp.tile([128, nk * 128], F32R, tag="wp")
                nc.sync.dma_start(wt[:], wdram[w])
                ps = psA.tile([128, BPC], F32, tag="psA")
                for k in range(nk):
                    nc.tensor.matmul(
                        ps[:],
                        wt[:, k * 128:(k + 1) * 128],
                        src[k][:],
                        start=(k == 0),
                        stop=(k == nk - 1),
                    )
                h = pool.tile([128, BPC], F32R, tag=tag)
                nc.scalar.activation(
                    h[:], ps[:], mybir.ActivationFunctionType.Relu,
                    bias=bt[:, li * WC + w: li * WC + w + 1],
                )
                outs.append(h)
            return outs

        h1 = trunk_layer(xt, w0, KC0, 0, actB, "actB")
        h2 = trunk_layer(h1, w1, KC, 1, actA, "actA")
        h3 = trunk_layer(h2, w2, KC, 2, actC, "actC")

        om = [op.tile([128, BPC], F32, tag="op", name=f"om{m}") for m in range(MH)]
        for s in range(S):
            hw = hwp.tile([128, KC * HEAD_DIM], F32R, tag="hwp")
            nc.sync.dma_start(hw[:], hws[s])
            for m in range(MH):
                ps = psB.tile([128, BPC], F32, tag="psB")
                for k in range(KC):
                    nc.tensor.matmul(
                        ps[:],
                        hw[:, k * HEAD_DIM + m * 128: k * HEAD_DIM + (m + 1) * 128],
                        h3[k][:],
                        start=(k == 0),
                        stop=(k == KC - 1),
                    )
                if s == 0:
                    nc.vector.tensor_copy(om[m][:], ps[:])
                else:
                    nc.vector.copy_predicated(
                        om[m][:], mt[:, s * BPC:(s + 1) * BPC], ps[:]
                    )
        for m in range(MH):
            nc.sync.dma_start(out[m], om[m][:])

    nc.compile()
    return nc


def _pack_trunk(W0, W1, W2, b0, b1, b2):
    W0pad = np.zeros((KPAD, WIDTH), np.float32)
    W0pad[:KIN] = round_fp32r(W0)
    W1 = round_fp32r(W1)
    W2 = round_fp32r(W2)
    w0p = np.ascontiguousarray(
        W0pad.reshape(KC0, 128, WC, 128).transpose(2, 1, 0, 3).reshape(WC, 128, KC0 * 128)
    )
    w1p = np.ascontiguousarray(
        W1.reshape(KC, 128, WC, 128).transpose(2, 1, 0, 3).reshape(WC, 128, KC * 128)
    )
    w2p = np.ascontiguousarray(
        W2.reshape(KC, 128, WC, 128).transpose(2, 1, 0, 3).reshape(WC, 128, KC * 128)
    )
    bias = np.zeros((128, 3 * WC), np.float32)
    for li, b in enumerate((b0, b1, b2)):
        bias[:, li * WC:(li + 1) * WC] = b.reshape(WC, 128).T
    return w0p, w1p, w2p, bias


def prepare(x, W0, b0, W1, b1, W2, b2, head_W, head_b):
    """Host-side sharding. Returns (in_maps, order, sorted_task_ids, S)."""
    x = np.asarray(x, np.float32)
    W0 = np.asarray(W0, np.float32)
    W1 = np.asarray(W1, np.float32)
    W2 = np.asarray(W2, np.float32)
    b0 = np.asarray(b0, np.float32)
    b1 = np.asarray(b1, np.float32)
    b2 = np.asarray(b2, np.float32)
    head_W = np.asarray(head_W, np.float32)

    tid = np.argmax(x[:, -NUM_TASKS:], axis=1)
    order = np.argsort(tid, kind="stable")
    x_s = x[order]
    t_s = tid[order]

    chunks = [t_s[c * BPC:(c + 1) * BPC] for c in range(NCORES)]
    tasks_per_core = [list(dict.fromkeys(ch.tolist())) for ch in chunks]
    S = max(len(t) for t in tasks_per_core)

    w0p, w1p, w2p, bias = _pack_trunk(W0, W1, W2, b0, b1, b2)
    head_W = round_fp32r(head_W)
    # hw_pack[t, kp, kc*256 + j] = head_W[t, kc*128 + kp, j]
    hw_pack = np.ascontiguousarray(
        head_W.reshape(NUM_TASKS, KC, 128, HEAD_DIM)
        .transpose(0, 2, 1, 3)
        .reshape(NUM_TASKS, 128, KC * HEAD_DIM)
    )

    in_maps = []
    for c in range(NCORES):
        xs = x_s[c * BPC:(c + 1) * BPC]
        xTp = np.zeros((KPAD, BPC), np.float32)
        xTp[:KIN] = round_fp32r(xs.T)
        tl = tasks_per_core[c]
        tl_p = tl + [tl[-1]] * (S - len(tl))
        hws_c = np.ascontiguousarray(hw_pack[np.asarray(tl_p)])
        msk_c = np.zeros((128, S * BPC), np.uint8)
        ch = chunks[c]
        for s, t in enumerate(tl):
            if s == 0:
                continue  # slot 0 is the tensor_copy base, mask unused
            msk_c[:, s * BPC:(s + 1) * BPC] = (ch == t)[None, :].astype(np.uint8)
        in_maps.append({
            "xT": np.ascontiguousarray(xTp.reshape(KC0, 128, BPC)),
            "w0p": w0p, "w1p": w1p, "w2p": w2p, "bias": bias,
            "hws": hws_c, "msk": msk_c,
        })
    return in_maps, order, t_s, S


def _assemble(results, order, t_s, head_b):
    head_b = np.asarray(head_b, np.float32)
    outs = []
    for c in range(NCORES):
        oT = results[c]["outT"]                       # [MH, 128, BPC]
        outs.append(oT.reshape(HEAD_DIM, BPC).T)      # [BPC, 256]
    out_s = np.concatenate(outs, axis=0) + head_b[t_s]
    out = np.empty_like(out_s)
    out[order] = out_s
    return out.astype(np.float32)


def kernel(x, W0, b0, W1, b1, W2, b2, head_W, head_b):
    in_maps, order, t_s, S = prepare(x, W0, b0, W1, b1, W2, b2, head_W, head_b)
    nc = _PROG_CACHE.get(S)
    if nc is None:
        nc = _build(S)
        _PROG_CACHE[S] = nc
    res = bass_utils.run_bass_kernel_spmd(nc, in_maps, core_ids=list(range(NCORES)))
    return _assemble(res.results, order, t_s, head_b)


# revision 6
# speedup vs baseline: 40334.6147x; 40334.6147x over previous
"""Trainium2 Bass kernel for nn_MultiHeadNetwork (moe_routing).

Strategy
--------
Host side (numpy, inside kernel()):
  * task id per row = argmax of the trailing one-hot block of x (data, not
    activation dependent), rows sorted by task id, batch split into 8
    contiguous 512-row chunks (one per NeuronCore).
  * Trunk weights replicated; per core the head only needs the few tasks its
    sorted chunk spans (<= S slots, S = max over cores), gathered on host.
  * All tensors are pre-packed so every DMA is a contiguous [128, F] panel.

Device side (one SPMD Tile program on 8 cores):
  * Activations kept feature-major (hT: [feat partitions, batch free]) so each
    trunk layer is out = W_chunk.T @ hT with NO transposes anywhere:
       lhsT = W[kc, wc] 128x128 (stationary), rhs = hT k-tile [128, 512].
  * Matmuls run as float32r (fp32 storage, full-rate PE for N >= 256).
  * ReLU + bias fused on the scalar engine straight out of PSUM.
  * Head: for each task slot s compute outT_s = head_W[t_s].T @ h3T for ALL
    512 columns (N=512 keeps the PE at full rate), then select the columns
    whose task == t_s via copy_predicated with a host-built 0/1 mask.
  * Head bias and the inverse permutation are applied on host.
"""

import numpy as np
from contextlib import ExitStack

import concourse.bacc as bacc
import concourse.mybir as mybir
from concourse.tile import TileContext
from concourse import bass_utils

BATCH = 4096
FEAT = 2048
NUM_TASKS = 50
WIDTH = 2048
HEAD_DIM = 256
NCORES = 8
BPC = BATCH // NCORES          # 512 rows per core
KIN = FEAT + NUM_TASKS         # 2098
KC0 = 17                       # input K chunks (zero-padded to 2176)
KPAD = KC0 * 128
KC = WIDTH // 128              # 16
WC = WIDTH // 128              # 16
MH = HEAD_DIM // 128           # 2 head-dim halves

F32 = mybir.dt.float32
F32R = mybir.dt.float32r

_PROG_CACHE: dict = {}


def round_fp32r(a: np.ndarray) -> np.ndarray:
    """Round fp32 to the fp32r grid (11 mantissa bits, RNE) like the HW does."""
    b = np.ascontiguousarray(a, np.float32).view(np.uint32)
    bias = np.uint32(0x7FF) + ((b >> np.uint32(12)) & np.uint32(1))
    out = (b + bias) & np.uint32(0xFFFFF000)
    return out.view(np.float32)


def _build(S: int, repeat: int = 1):
    """Build + compile the SPMD Tile program for S head slots per core.

    repeat > 1 wraps the whole body in a hardware For_i loop (benchmarking
    only: amortizes launch/RPC overhead across repeat executions).
    """
    nc = bacc.Bacc("TRN2", target_bir_lowering=False, debug=False)
    xT = nc.dram_tensor("xT", [KC0, 128, BPC], F32R, kind="ExternalInput").ap()
    w0 = nc.dram_tensor("w0p", [WC, 128, KC0 * 128], F32R, kind="ExternalInput").ap()
    w1 = nc.dram_tensor("w1p", [WC, 128, KC * 128], F32R, kind="ExternalInput").ap()
    w2 = nc.dram_tensor("w2p", [WC, 128, KC * 128], F32R, kind="ExternalInput").ap()
    bia = nc.dram_tensor("bias", [128, 3 * WC], F32, kind="ExternalInput").ap()
    hws = nc.dram_tensor("hws", [S, 128, KC * HEAD_DIM], F32R, kind="ExternalInput").ap()
    msk = nc.dram_tensor("msk", [128, S * BPC], mybir.dt.uint8, kind="ExternalInput").ap()
    out = nc.dram_tensor("outT", [MH, 128, BPC], F32, kind="ExternalOutput").ap()

    with TileContext(nc) as tc, ExitStack() as ctx:
        # xT (17 tiles) and h2 (16) share slots: h2 allocates only after
        # layer 0 fully finished reading xT.  Same for h1 vs nothing; h3 gets
        # its own pool (h1 is still being read while h3 is produced).
        actA = ctx.enter_context(tc.tile_pool(name="actA", bufs=KC0))
        actB = ctx.enter_context(tc.tile_pool(name="actB", bufs=KC))
        actC = ctx.enter_context(tc.tile_pool(name="actC", bufs=KC))
        wp = ctx.enter_context(tc.tile_pool(name="wp", bufs=3))
        cons = ctx.enter_context(tc.tile_pool(name="cons", bufs=1))
        hwp = ctx.enter_context(tc.tile_pool(name="hwp", bufs=2))
        op = ctx.enter_context(tc.tile_pool(name="op", bufs=MH))
        psA = ctx.enter_context(tc.tile_pool(name="psA", bufs=4, space="PSUM"))
        psB = ctx.enter_context(tc.tile_pool(name="psB", bufs=4, space="PSUM"))

        if repeat > 1:
            ctx.enter_context(tc.For_i(0, repeat, 1))

        bt = cons.tile([128, 3 * WC], F32, tag="bt")
        nc.sync.dma_start(bt[:], bia)
        mt = cons.tile([128, S * BPC], mybir.dt.uint8, tag="mt")
        nc.sync.dma_start(mt[:], msk)

        xt = []
        for k in range(KC0):
            t = actA.tile([128, BPC], F32R, tag="actA")
            nc.sync.dma_start(t[:], xT[k])
            xt.append(t)

        def trunk_layer(src, wdram, nk, li, pool, tag):
            outs = []
            for w in range(WC):
                wt = wp.tile([128, nk * 128], F32R, tag="wp")
                nc.sync.dma_start(wt[:], wdram[w])
                ps = psA.tile([128, BPC], F32, tag="psA")
                for k in range(nk):
                    nc.tensor.matmul(
                        ps[:],
                        wt[:, k * 128:(k + 1) * 128],
                        src[k][:],
                        start=(k == 0),
                        stop=(k == nk - 1),
                    )
                h = pool.tile([128, BPC], F32R, tag=tag)
                nc.scalar.activation(
                    h[:], ps[:], mybir.ActivationFunctionType.Relu,
                    bias=bt[:, li * WC + w: li * WC + w + 1],
                )
                outs.append(h)
            return outs

        h1 = trunk_layer(xt, w0, KC0, 0, actB, "actB")
        h2 = trunk_layer(h1, w1, KC, 1, actA, "actA")
        h3 = trunk_layer(h2, w2, KC, 2, actC, "actC")

        om = [op.tile([128, BPC], F32, tag="op", name=f"om{m}") for m in range(MH)]
        for s in range(S):
            hw = hwp.tile([128, KC * HEAD_DIM], F32R, tag="hwp")
            nc.sync.dma_start(hw[:], hws[s])
            for m in range(MH):
                ps = psB.tile([128, BPC], F32, tag="psB")
                for k in range(KC):
                    nc.tensor.matmul(
                        ps[:],
                        hw[:, k * HEAD_DIM + m * 128: k * HEAD_DIM + (m + 1) * 128],
                        h3[k][:],
                        start=(k == 0),
                        stop=(k == KC - 1),
                    )
                if s == 0:
                    nc.vector.tensor_copy(om[m][:], ps[:])
                else:
                    nc.vector.copy_predicated(
                        om[m][:], mt[:, s * BPC:(s + 1) * BPC], ps[:]
                    )
        for m in range(MH):
            nc.sync.dma_start(out[m], om[m][:])

    nc.compile()
    return nc


def _pack_trunk(W0, W1, W2, b0, b1, b2):
    W0pad = np.zeros((KPAD, WIDTH), np.float32)
    W0pad[:KIN] = round_fp32r(W0)
    W1 = round_fp32r(W1)
    W2 = round_fp32r(W2)
    w0p = np.ascontiguousarray(
        W0pad.reshape(KC0, 128, WC, 128).transpose(2, 1, 0, 3).reshape(WC, 128, KC0 * 128)
    )
    w1p = np.ascontiguousarray(
        W1.reshape(KC, 128, WC, 128).transpose(2, 1, 0, 3).reshape(WC, 128, KC * 128)
    )
    w2p = np.ascontiguousarray(
        W2.reshape(KC, 128, WC, 128).transpose(2, 1, 0, 3).reshape(WC, 128, KC * 128)
    )
    bias = np.zeros((128, 3 * WC), np.float32)
    for li, b in enumerate((b0, b1, b2)):
        bias[:, li * WC:(li + 1) * WC] = b.reshape(WC, 128).T
    return w0p, w1p, w2p, bias


def prepare(x, W0, b0, W1, b1, W2, b2, head_W, head_b):
    """Host-side sharding. Returns (in_maps, order, sorted_task_ids, S)."""
    x = np.asarray(x, np.float32)
    W0 = np.asarray(W0, np.float32)
    W1 = np.asarray(W1, np.float32)
    W2 = np.asarray(W2, np.float32)
    b0 = np.asarray(b0, np.float32)
    b1 = np.asarray(b1, np.float32)
    b2 = np.asarray(b2, np.float32)
    head_W = np.asarray(head_W, np.float32)

    tid = np.argmax(x[:, -NUM_TASKS:], axis=1)
    order = np.argsort(tid, kind="stable")
    x_s = x[order]
    t_s = tid[order]

    chunks = [t_s[c * BPC:(c + 1) * BPC] for c in range(NCORES)]
    tasks_per_core = [list(dict.fromkeys(ch.tolist())) for ch in chunks]
    S = max(len(t) for t in tasks_per_core)

    w0p, w1p, w2p, bias = _pack_trunk(W0, W1, W2, b0, b1, b2)
    head_W = round_fp32r(head_W)
    # hw_pack[t, kp, kc*256 + j] = head_W[t, kc*128 + kp, j]
    hw_pack = np.ascontiguousarray(
        head_W.reshape(NUM_TASKS, KC, 128, HEAD_DIM)
        .transpose(0, 2, 1, 3)
        .reshape(NUM_TASKS, 128, KC * HEAD_DIM)
    )

    in_maps = []
    for c in range(NCORES):
        xs = x_s[c * BPC:(c + 1) * BPC]
        xTp = np.zeros((KPAD, BPC), np.float32)
        xTp[:KIN] = round_fp32r(xs.T)
        tl = tasks_per_core[c]
        tl_p = tl + [tl[-1]] * (S - len(tl))
        hws_c = np.ascontiguousarray(hw_pack[np.asarray(tl_p)])
        msk_c = np.zeros((128, S * BPC), np.uint8)
        ch = chunks[c]
        for s, t in enumerate(tl):
            if s == 0:
                continue  # slot 0 is the tensor_copy base, mask unused
            msk_c[:, s * BPC:(s + 1) * BPC] = (ch == t)[None, :].astype(np.uint8)
        in_maps.append({
            "xT": np.ascontiguousarray(xTp.reshape(KC0, 128, BPC)),
            "w0p": w0p, "w1p": w1p, "w2p": w2p, "bias": bias,
            "hws": hws_c, "msk": msk_c,
        })
    return in_maps, order, t_s, S


def _assemble(results, order, t_s, head_b):
    head_b = np.asarray(head_b, np.float32)
    outs = []
    for c in range(NCORES):
        oT = results[c]["outT"]                       # [MH, 128, BPC]
        outs.append(oT.reshape(HEAD_DIM, BPC).T)      # [BPC, 256]
    out_s = np.concatenate(outs, axis=0) + head_b[t_s]
    out = np.empty_like(out_s)
    out[order] = out_s
    return out.astype(np.float32)


def kernel(x, W0, b0, W1, b1, W2, b2, head_W, head_b):
    in_maps, order, t_s, S = prepare(x, W0, b0, W1, b1, W2, b2, head_W, head_b)
    nc = _PROG_CACHE.get(S)
    if nc is None:
        nc = _build(S)
        _PROG_CACHE[S] = nc
    res = bass_utils.run_bass_kernel_spmd(nc, in_maps, core_ids=list(range(NCORES)))
    return _assemble(res.results, order, t_s, head_b)
